# revision 34
# baseline (speedup 1.0000x reference)
"""Quantized int8 3x3 conv (dequant -> conv -> requant) on 8 TRN2 NeuronCores.

Sharding: data-parallel over batch (16 images -> 2 per core), weights/bias
replicated.  No cross-core communication.

v3: parity-quadrant decomposition.  The 128x128 PE array is addressed as
four 64x64 quadrants via tile_position; 4 quadrant matmuls (K=64, M=64,
N=512) run concurrently at the SAME sustained cadence as one full-array
matmul (HW-measured 215.8ns/round, bench_quad.py).  This removes the v2
scheme's 25% structural-zero waste (its 2-row K-blocks held 6 useful taps
in 8 slots), cutting the matmul stream from 762 to ~571 512-col rounds.

Per 4-row unit (rows y0..y0+3), rows pair with quadrants so that every
quadrant queue is exactly 9 MMs (one per (kh,kw) tap) and every psum
slice is written by a single quadrant (a slice accumulated from two
tile_positions faults the device -- see bench_quad.py 'handoff'):
  - quadrant (rg, cg): the kh0+kh2 taps of the cg-pair row whose parity
    is rg (6 MMs, into its "home" partial bank) plus the sibling row's
    kh1 taps (3 MMs, into that row's "other" partial bank).  The rhs
    partition half always equals rg (walrus requires fmap and weights to
    start at the same partition), which the packed input layout (in-row
    parity = partition half) provides with no extra DMA traffic.
  - per unit 4 partial banks; ACT merges home partials (psum*scale+bias,
    per-partition bias AP) into SBUF, DVE fuses (other*scale)+merged ->
    int16.  One [128,512] op per engine per output-row pair.
  - the final 2 rows run as one v2-style full-array pair (6 K=128 MMs,
    single tensor_scalar requant) for a shorter post-stream drain.

Math (exact-integer in disguise): (x-7) and (w-3) are 8/9-bit ints, exact
in bf16; products < 2^16 and psum partial sums < 2^24 are exact in fp32.
"""

import numpy as np
import ml_dtypes

import concourse.bass as bass
import concourse.tile as tile
from concourse import bacc, mybir
from concourse.bass_utils import run_bass_kernel_spmd

N_CORES = 8
IN_ZP = 7
W_ZP = 3
SCALE = 1e-4  # IN_SCALE * W_SCALE; OUT_SCALE=1, OUT_ZP=0, B_SCALE=1, B_ZP=0
BF16 = ml_dtypes.bfloat16

H = W = 256
C = CO = 64
OH = OW = H - 2
N_IMG = 2                 # images per core
BLK = N_IMG * W           # 512 free-dim columns per in-row block
N_GROUPS = H // 8         # 32 input groups of 8 rows (4 blocks x 2 parity)
GCOL = 4 * BLK            # 2048
UNIT_ROWS = 4
N_UNITS = (OH + UNIT_ROWS - 1) // UNIT_ROWS   # 64 (63 full + tail of 2)


def build_nc(n_cores=N_CORES):
    nc = bacc.Bacc("TRN2", target_bir_lowering=False, debug=False,
                   num_devices=n_cores)
    xp = nc.declare_dram_parameter("xp", [N_GROUPS, 128, GCOL],
                                   mybir.dt.bfloat16, isOutput=False)
    # 9 quadrant taps (64 cols each) + 6 v2-style full-array tiles for the
    # tail pair (128 cols each)
    wp = nc.declare_dram_parameter("wp", [128, 9 * 64 + 6 * 128],
                                   mybir.dt.bfloat16, isOutput=False)
    bp = nc.declare_dram_parameter("bp", [128, 1], mybir.dt.float32,
                                   isOutput=False)
    yp = nc.declare_dram_parameter("yp", [N_UNITS, 128, 2 * BLK],
                                   mybir.dt.int16, isOutput=True)

    with tile.TileContext(nc) as tc:
        with (
            tc.tile_pool(name="const", bufs=1) as constp,
            tc.tile_pool(name="x2", bufs=6) as x2p,
            tc.tile_pool(name="outp", bufs=4) as outp,
            tc.tile_pool(name="tmp", bufs=4) as tmpp,
            tc.tile_pool(name="psum", bufs=8, space="PSUM") as psp,
        ):
            x2_tiles = {}

            def load_group(g, eng=None):
                t = x2p.tile([128, GCOL], mybir.dt.bfloat16, tag="x2")
                if eng is None:
                    eng = nc.scalar
                eng.dma_start(t[:], xp[g])
                x2_tiles[g] = t

            # HAM pre-warm: dummy PE activity bridging the engine preamble
            # and the first input data landing.  The activity window only
            # flips to full clock after a FULLY busy ~3.4us window, so the
            # warmup must run gap-free into the real stream.
            warm = constp.tile([128, 512], mybir.dt.bfloat16, tag="warm")
            nc.vector.memset(warm[:], 0.0)
            warm_ps = psp.tile([128, BLK], mybir.dt.float32, tag="ps",
                               name="warm_ps")
            # N=512 warmup MMs (N=128 ones leave too much per-MM issue idle
            # to count as 'fully busy' for the HAM window): HAM fires after
            # ~8, the last ones run warm and bridge to the input data; 12
            # covers late data so the stream never restarts the busy window
            for _ in range(12):
                nc.tensor.matmul(warm_ps[:], warm[:, 0:128], warm[:],
                                 start=True, stop=True)

            # prologue: tiny weight/bias loads drain first on each HWDGE
            # queue, then the first group load is split across both queues
            wt = constp.tile([128, 9 * 64 + 6 * 128],
                             mybir.dt.bfloat16, tag="wt")
            nc.scalar.dma_start(wt[:], wp[:])
            bias_f = constp.tile([128, 1], mybir.dt.float32, tag="bias_f")
            nc.sync.dma_start(bias_f[:], bp[:])

            g0 = x2p.tile([128, GCOL], mybir.dt.bfloat16, tag="x2")
            nc.scalar.dma_start(g0[:, 0:GCOL // 2], xp[0][:, 0:GCOL // 2])
            nc.sync.dma_start(g0[:, GCOL // 2:], xp[0][:, GCOL // 2:])
            x2_tiles[0] = g0
            load_group(1, nc.scalar)

            def rhs_slice(y, kh, kw, width):
                r = y + kh
                g, lb, half = r // 8, (r % 8) // 2, r % 2
                t = x2_tiles[g]
                return t[half * 64:half * 64 + 64,
                         lb * BLK + kw:lb * BLK + kw + width]

            def tail_pair_unit(u):
                # final 2 rows as one v2-style full-array pair: 6 K=128 MMs
                # into one bank, single-op requant, two half-size stores --
                # a shorter post-stream drain than a 9-round quadrant unit.
                ps = psp.tile([128, BLK], mybir.dt.float32, tag="ps",
                              name=f"ps_{u}_t")
                pair = 2 * u  # pair index in v2 layout (rows 4u, 4u+1)
                for j2 in range(2):
                    for kw in range(3):
                        t6 = j2 * 3 + kw
                        lt = wt[:, 576 + t6 * 128:576 + (t6 + 1) * 128]
                        first = (j2 == 0 and kw == 0)
                        width = BLK if first else BLK - 2
                        g2, lb = divmod(pair + j2, 4)
                        rhs = x2_tiles[g2][:, lb * BLK + kw:
                                           lb * BLK + kw + width]
                        nc.tensor.matmul(ps[:, 0:width], lt, rhs,
                                         start=first,
                                         stop=(j2 == 1 and kw == 2))
                ot = outp.tile([128, BLK], mybir.dt.int16, tag="out")
                nc.vector.tensor_scalar(ot[:], ps[:], SCALE, bias_f[:],
                                        mybir.AluOpType.mult,
                                        mybir.AluOpType.add)
                # v2 bank layout: partition r*64+o (r = row in pair); the
                # yp layout wants row 4u+2*cg+b at (cg*64+o, b*BLK+...)
                nc.sync.dma_start(yp[u][0:64, 0:BLK], ot[0:64, :])
                nc.sync.dma_start(yp[u][0:64, BLK:2 * BLK], ot[64:128, :])

            def compute_unit(u):
                rows = [4 * u + i for i in range(UNIT_ROWS) if 4 * u + i < OH]
                if len(rows) == 2:
                    tail_pair_unit(u)
                    return
                # 4 partial banks, each written by a SINGLE row-group (an
                # accumulation slice written from two tile_positions faults
                # the device -- bench_quad 'handoff' repro):
                #   A: kh0+kh2 partials of even rows (y0,y2)   <- rg0
                #   B: kh0+kh2 partials of odd rows (y1,y3)    <- rg1
                #   C: kh1 partials of even rows               <- rg1
                #   D: kh1 partials of odd rows                <- rg0
                # gpsimd then sums A+C (rows y0,y2) and B+D (y1,y3) and the
                # DVE requant reads the sums, keeping DVE load unchanged.
                A, B, Cc, D = [psp.tile([128, BLK], mybir.dt.float32,
                                        tag="ps", name=f"ps_{u}_{b}")
                               for b in range(4)]
                Q = {(rg, cg): [] for rg in (0, 1) for cg in (0, 1)}
                for r4, y in enumerate(rows):
                    cg, p = r4 // 2, y % 2
                    home = A if p == 0 else B
                    other = Cc if p == 0 else D
                    for kh in (0, 2):
                        for kw in range(3):
                            Q[(p, cg)].append((home, y, kh, kw))
                    for kw in range(3):
                        Q[(1 - p, cg)].append((other, y, 1, kw))
                for rnd in range(9):
                    for (rg, cg) in ((0, 0), (1, 0), (0, 1), (1, 1)):
                        q = Q[(rg, cg)]
                        if rnd >= len(q):
                            continue
                        bank, y, kh, kw = q[rnd]
                        first = (kw == 0 and kh != 2)   # kh0kw0 / kh1kw0
                        width = BLK if first else BLK - 2
                        lhsT = wt[rg * 64:rg * 64 + 64,
                                  (kh * 3 + kw) * 64:(kh * 3 + kw) * 64 + 64]
                        nc.tensor.matmul(
                            bank[cg * 64:cg * 64 + 64, 0:width],
                            lhsT, rhs_slice(y, kh, kw, width),
                            start=first,
                            stop=(kw == 2 and kh != 0),  # kh2kw2 / kh1kw2
                            skip_group_check=True,
                            tile_position=(rg * 64, cg * 64))
                ot = outp.tile([128, 2 * BLK], mybir.dt.int16, tag="out")
                for b, (hb, ob) in enumerate(((A, Cc), (B, D))):
                    # ACT: tmp = home*SCALE + bias (reads PSUM, idle engine);
                    # DVE: ot = (other*SCALE) + tmp -> int16
                    tmp = tmpp.tile([128, BLK], mybir.dt.float32, tag="tmp")
                    nc.scalar.activation(
                        tmp[:], hb[:], mybir.ActivationFunctionType.Identity,
                        bias=bias_f[:], scale=float(SCALE))
                    nc.vector.scalar_tensor_tensor(
                        ot[:, b * BLK:(b + 1) * BLK], ob[:], float(SCALE),
                        tmp[:], mybir.AluOpType.mult, mybir.AluOpType.add)
                nc.sync.dma_start(yp[u], ot[:])

            for u in range(N_UNITS):
                compute_unit(u)
                if u % 2 == 1:
                    g = u // 2 + 2
                    if g < N_GROUPS:
                        load_group(g)

    _dedup_ldweights(nc)
    nc.compile()
    return nc


def _dedup_ldweights(nc):
    """Remove InstLdweights that reload the weights already in the PE array.

    tile_legalize pairs EVERY non-self-loading InstMatmult with its own
    InstLdweights.  The PE array is weight-stationary: one load per weight
    change suffices.  Keep the first load of each distinct weights AP, drop
    consecutive repeats (only ever legalize-inserted ones, which have no
    descendants).
    """
    for f in nc.m.functions:
        for b in f.blocks:
            insts = b.instructions
            loaded = None
            drop = []
            for idx, i in enumerate(insts):
                nm = type(i).__name__
                if nm == 'InstLdweights':
                    sig = (i.ins[0].concise(), str(i.tile_position),
                           str(i.tile_size))
                    if sig == loaded and not list(i.descendants or []):
                        drop.append(idx)
                    else:
                        loaded = sig
                elif nm == 'InstMatmult':
                    assert not i.ldweights, i.name
            for idx in reversed(drop):
                del insts[idx]


_NC_CACHE = {}


def get_nc(*_args, **_kwargs):
    if "nc" not in _NC_CACHE:
        _NC_CACHE["nc"] = build_nc()
    return _NC_CACHE["nc"]


def pack_inputs(input, weight, bias):
    """Host-side prepack: returns per-core in_maps."""
    x = np.ascontiguousarray(input, dtype=np.int32)
    # [core, img, ch, g, b, par, w] -> [core, g, par, ch, b, img, w]
    xr = (x.astype(np.int16) - IN_ZP).astype(BF16)
    xr = xr.reshape(N_CORES, N_IMG, C, N_GROUPS, 4, 2, W)
    xr = np.ascontiguousarray(xr.transpose(0, 3, 5, 2, 4, 1, 6))
    xp = xr.reshape(N_CORES, N_GROUPS, 128, GCOL)

    wf = weight.astype(np.float32) - W_ZP            # [O, I, kh, kw]
    # wp[p, (kh*3+kw)*64 + o] = wf[o, p%64, kh, kw], both partition halves
    wpk = np.zeros((128, 9 * 64 + 6 * 128), np.float32)
    for kh in range(3):
        for kw in range(3):
            t = kh * 3 + kw
            blk = wf[:, :, kh, kw].T                 # [ci, o]
            wpk[0:64, t * 64:(t + 1) * 64] = blk
            wpk[64:128, t * 64:(t + 1) * 64] = blk
    # v2-style full-array tiles for the tail pair:
    # lhs[t][par*64+c][r*64+o] = wf[o, c, 2*j2+par-r, kw]
    lhs = np.zeros((6, 128, 128), np.float32)
    for j2 in range(2):
        for kw in range(3):
            t = j2 * 3 + kw
            for par in range(2):
                for r in range(2):
                    kh = 2 * j2 + par - r
                    if 0 <= kh <= 2:
                        lhs[t, par * 64:par * 64 + 64,
                            r * 64:r * 64 + 64] = wf[:, :, kh, kw].T
    wpk[:, 576:] = lhs.transpose(1, 0, 2).reshape(128, 768)
    wpk = wpk.astype(BF16)

    bpk = np.concatenate([bias, bias]).astype(np.float32).reshape(128, 1)

    return [{"xp": np.ascontiguousarray(xp[i]), "wp": wpk, "bp": bpk}
            for i in range(N_CORES)]


def unpack_output(yp):
    """[N_UNITS, 128, 2*BLK] int16 -> [N_IMG, CO, OH, OW] int32.

    yp[u, cg*64+co, bank*BLK + img*W + w] = out row 4u + 2*cg + bank.
    """
    a = yp.reshape(N_UNITS, 2, CO, 2, N_IMG, W)      # [u, cg, co, bank, img, w]
    a = a.transpose(4, 2, 0, 1, 3, 5)                # [img, co, u, cg, bank, w]
    a = a.reshape(N_IMG, CO, N_UNITS * 4, W)
    return a[:, :, :OH, :OW].astype(np.int32)


def run_sharded(nc, input, weight, bias, n_img=N_IMG, **kwargs):
    in_maps = pack_inputs(input, weight, bias)
    res = run_bass_kernel_spmd(nc, in_maps, list(range(N_CORES)), **kwargs)
    out = np.concatenate([unpack_output(r["yp"]) for r in res.results],
                         axis=0)
    return out, res


def kernel(input, weight, bias):
    nc = get_nc()
    out, _ = run_sharded(nc, input, weight, bias)
    return out


# revision 35
# speedup vs baseline: 1.0235x; 1.0235x over previous
"""Quantized int8 3x3 conv (dequant -> conv -> requant) on 8 TRN2 NeuronCores.

Sharding: data-parallel over batch (16 images -> 2 per core), weights/bias
replicated.  No cross-core communication.

v3: parity-quadrant decomposition.  The 128x128 PE array is addressed as
four 64x64 quadrants via tile_position; 4 quadrant matmuls (K=64, M=64,
N=512) run concurrently at the SAME sustained cadence as one full-array
matmul (HW-measured 215.8ns/round, bench_quad.py).  This removes the v2
scheme's 25% structural-zero waste (its 2-row K-blocks held 6 useful taps
in 8 slots), cutting the matmul stream from 762 to ~571 512-col rounds.

Per 4-row unit (rows y0..y0+3), rows pair with quadrants so that every
quadrant queue is exactly 9 MMs (one per (kh,kw) tap) and every psum
slice is written by a single quadrant (a slice accumulated from two
tile_positions faults the device -- see bench_quad.py 'handoff'):
  - quadrant (rg, cg): the kh0+kh2 taps of the cg-pair row whose parity
    is rg (6 MMs, into its "home" partial bank) plus the sibling row's
    kh1 taps (3 MMs, into that row's "other" partial bank).  The rhs
    partition half always equals rg (walrus requires fmap and weights to
    start at the same partition), which the packed input layout (in-row
    parity = partition half) provides with no extra DMA traffic.
  - per unit 4 partial banks; ACT merges home partials (psum*scale+bias,
    per-partition bias AP) into SBUF, DVE fuses (other*scale)+merged ->
    int16.  One [128,512] op per engine per output-row pair.
  - the final 2 rows run as one v2-style full-array pair (6 K=128 MMs,
    single tensor_scalar requant) for a shorter post-stream drain.

Math (exact-integer in disguise): (x-7) and (w-3) are 8/9-bit ints, exact
in bf16; products < 2^16 and psum partial sums < 2^24 are exact in fp32.
"""

import numpy as np
import ml_dtypes

import concourse.bass as bass
import concourse.tile as tile
from concourse import bacc, mybir
from concourse.bass_utils import run_bass_kernel_spmd

N_CORES = 8
IN_ZP = 7
W_ZP = 3
SCALE = 1e-4  # IN_SCALE * W_SCALE; OUT_SCALE=1, OUT_ZP=0, B_SCALE=1, B_ZP=0
BF16 = ml_dtypes.bfloat16

H = W = 256
C = CO = 64
OH = OW = H - 2
N_IMG = 2                 # images per core
BLK = N_IMG * W           # 512 free-dim columns per in-row block
N_GROUPS = H // 8         # 32 input groups of 8 rows (4 blocks x 2 parity)
GCOL = 4 * BLK            # 2048
UNIT_ROWS = 4
N_UNITS = (OH + UNIT_ROWS - 1) // UNIT_ROWS   # 64 (63 full + tail of 2)


def build_nc(n_cores=N_CORES):
    nc = bacc.Bacc("TRN2", target_bir_lowering=False, debug=False,
                   num_devices=n_cores)
    xp = nc.declare_dram_parameter("xp", [N_GROUPS, 128, GCOL],
                                   mybir.dt.bfloat16, isOutput=False)
    # 9 quadrant taps (64 cols each) + 6 v2-style full-array tiles for the
    # tail pair (128 cols each)
    wp = nc.declare_dram_parameter("wp", [128, 9 * 64 + 6 * 128],
                                   mybir.dt.bfloat16, isOutput=False)
    bp = nc.declare_dram_parameter("bp", [128, 1], mybir.dt.float32,
                                   isOutput=False)
    yp = nc.declare_dram_parameter("yp", [N_UNITS, 128, 2 * BLK],
                                   mybir.dt.int16, isOutput=True)

    with tile.TileContext(nc) as tc:
        with (
            tc.tile_pool(name="const", bufs=1) as constp,
            tc.tile_pool(name="x2", bufs=6) as x2p,
            tc.tile_pool(name="outp", bufs=4) as outp,
            tc.tile_pool(name="tmp", bufs=4) as tmpp,
            tc.tile_pool(name="psum", bufs=8, space="PSUM") as psp,
        ):
            x2_tiles = {}

            def load_group(g, eng=None):
                t = x2p.tile([128, GCOL], mybir.dt.bfloat16, tag="x2")
                if eng is None:
                    eng = nc.scalar
                eng.dma_start(t[:], xp[g])
                x2_tiles[g] = t

            # HAM pre-warm: dummy PE activity bridging the engine preamble
            # and the first input data landing.  The activity window only
            # flips to full clock after a FULLY busy ~3.4us window, so the
            # warmup must run gap-free into the real stream.
            warm = constp.tile([128, 512], mybir.dt.bfloat16, tag="warm")
            # gpsimd clears the engine barrier ~0.5us before the DVE, so
            # the memset (and with it the first warmup MM) starts earlier
            nc.gpsimd.memset(warm[:], 0.0)
            warm_ps = psp.tile([128, BLK], mybir.dt.float32, tag="ps",
                               name="warm_ps")
            # N=512 warmup MMs (N=128 ones leave too much per-MM issue idle
            # to count as 'fully busy' for the HAM window): HAM fires after
            # ~8, the last ones run warm and bridge to the input data; 12
            # covers late data so the stream never restarts the busy window
            for _ in range(14):
                nc.tensor.matmul(warm_ps[:], warm[:, 0:128], warm[:],
                                 start=True, stop=True)

            # prologue: tiny weight/bias loads drain first on each HWDGE
            # queue, then the first group load is split across both queues
            wt = constp.tile([128, 9 * 64 + 6 * 128],
                             mybir.dt.bfloat16, tag="wt")
            nc.scalar.dma_start(wt[:], wp[:])
            bias_f = constp.tile([128, 1], mybir.dt.float32, tag="bias_f")
            nc.sync.dma_start(bias_f[:], bp[:])

            g0 = x2p.tile([128, GCOL], mybir.dt.bfloat16, tag="x2")
            nc.scalar.dma_start(g0[:, 0:GCOL // 2], xp[0][:, 0:GCOL // 2])
            nc.sync.dma_start(g0[:, GCOL // 2:], xp[0][:, GCOL // 2:])
            x2_tiles[0] = g0
            load_group(1, nc.scalar)

            def rhs_slice(y, kh, kw, width):
                r = y + kh
                g, lb, half = r // 8, (r % 8) // 2, r % 2
                t = x2_tiles[g]
                return t[half * 64:half * 64 + 64,
                         lb * BLK + kw:lb * BLK + kw + width]

            def tail_pair_unit(u):
                # final 2 rows as one v2-style full-array pair: 6 K=128 MMs
                # into one bank, single-op requant, two half-size stores --
                # a shorter post-stream drain than a 9-round quadrant unit.
                ps = psp.tile([128, BLK], mybir.dt.float32, tag="ps",
                              name=f"ps_{u}_t")
                pair = 2 * u  # pair index in v2 layout (rows 4u, 4u+1)
                for j2 in range(2):
                    for kw in range(3):
                        t6 = j2 * 3 + kw
                        lt = wt[:, 576 + t6 * 128:576 + (t6 + 1) * 128]
                        first = (j2 == 0 and kw == 0)
                        width = BLK if first else BLK - 2
                        g2, lb = divmod(pair + j2, 4)
                        rhs = x2_tiles[g2][:, lb * BLK + kw:
                                           lb * BLK + kw + width]
                        nc.tensor.matmul(ps[:, 0:width], lt, rhs,
                                         start=first,
                                         stop=(j2 == 1 and kw == 2))
                ot = outp.tile([128, BLK], mybir.dt.int16, tag="out")
                nc.vector.tensor_scalar(ot[:], ps[:], SCALE, bias_f[:],
                                        mybir.AluOpType.mult,
                                        mybir.AluOpType.add)
                # v2 bank layout: partition r*64+o (r = row in pair); the
                # yp layout wants row 4u+2*cg+b at (cg*64+o, b*BLK+...)
                nc.sync.dma_start(yp[u][0:64, 0:BLK], ot[0:64, :])
                nc.sync.dma_start(yp[u][0:64, BLK:2 * BLK], ot[64:128, :])

            def compute_unit(u):
                rows = [4 * u + i for i in range(UNIT_ROWS) if 4 * u + i < OH]
                if len(rows) == 2:
                    tail_pair_unit(u)
                    return
                # 4 partial banks, each written by a SINGLE row-group (an
                # accumulation slice written from two tile_positions faults
                # the device -- bench_quad 'handoff' repro):
                #   A: kh0+kh2 partials of even rows (y0,y2)   <- rg0
                #   B: kh0+kh2 partials of odd rows (y1,y3)    <- rg1
                #   C: kh1 partials of even rows               <- rg1
                #   D: kh1 partials of odd rows                <- rg0
                # gpsimd then sums A+C (rows y0,y2) and B+D (y1,y3) and the
                # DVE requant reads the sums, keeping DVE load unchanged.
                A, B, Cc, D = [psp.tile([128, BLK], mybir.dt.float32,
                                        tag="ps", name=f"ps_{u}_{b}")
                               for b in range(4)]
                Q = {(rg, cg): [] for rg in (0, 1) for cg in (0, 1)}
                for r4, y in enumerate(rows):
                    cg, p = r4 // 2, y % 2
                    home = A if p == 0 else B
                    other = Cc if p == 0 else D
                    for kh in (0, 2):
                        for kw in range(3):
                            Q[(p, cg)].append((home, y, kh, kw))
                    for kw in range(3):
                        Q[(1 - p, cg)].append((other, y, 1, kw))
                for rnd in range(9):
                    for (rg, cg) in ((0, 0), (1, 0), (0, 1), (1, 1)):
                        q = Q[(rg, cg)]
                        if rnd >= len(q):
                            continue
                        bank, y, kh, kw = q[rnd]
                        first = (kw == 0 and kh != 2)   # kh0kw0 / kh1kw0
                        width = BLK if first else BLK - 2
                        lhsT = wt[rg * 64:rg * 64 + 64,
                                  (kh * 3 + kw) * 64:(kh * 3 + kw) * 64 + 64]
                        nc.tensor.matmul(
                            bank[cg * 64:cg * 64 + 64, 0:width],
                            lhsT, rhs_slice(y, kh, kw, width),
                            start=first,
                            stop=(kw == 2 and kh != 0),  # kh2kw2 / kh1kw2
                            skip_group_check=True,
                            tile_position=(rg * 64, cg * 64))
                ot = outp.tile([128, 2 * BLK], mybir.dt.int16, tag="out")
                for b, (hb, ob) in enumerate(((A, Cc), (B, D))):
                    # ACT: tmp = home*SCALE + bias (reads PSUM, idle engine);
                    # DVE: ot = (other*SCALE) + tmp -> int16
                    tmp = tmpp.tile([128, BLK], mybir.dt.float32, tag="tmp")
                    nc.scalar.activation(
                        tmp[:], hb[:], mybir.ActivationFunctionType.Identity,
                        bias=bias_f[:], scale=float(SCALE))
                    nc.vector.scalar_tensor_tensor(
                        ot[:, b * BLK:(b + 1) * BLK], ob[:], float(SCALE),
                        tmp[:], mybir.AluOpType.mult, mybir.AluOpType.add)
                nc.sync.dma_start(yp[u], ot[:])

            for u in range(N_UNITS):
                compute_unit(u)
                if u % 2 == 1:
                    g = u // 2 + 2
                    if g < N_GROUPS:
                        load_group(g)

    _dedup_ldweights(nc)
    nc.compile()
    return nc


def _dedup_ldweights(nc):
    """Remove InstLdweights that reload the weights already in the PE array.

    tile_legalize pairs EVERY non-self-loading InstMatmult with its own
    InstLdweights.  The PE array is weight-stationary: one load per weight
    change suffices.  Keep the first load of each distinct weights AP, drop
    consecutive repeats (only ever legalize-inserted ones, which have no
    descendants).
    """
    for f in nc.m.functions:
        for b in f.blocks:
            insts = b.instructions
            loaded = None
            drop = []
            for idx, i in enumerate(insts):
                nm = type(i).__name__
                if nm == 'InstLdweights':
                    sig = (i.ins[0].concise(), str(i.tile_position),
                           str(i.tile_size))
                    if sig == loaded and not list(i.descendants or []):
                        drop.append(idx)
                    else:
                        loaded = sig
                elif nm == 'InstMatmult':
                    assert not i.ldweights, i.name
            for idx in reversed(drop):
                del insts[idx]


_NC_CACHE = {}


def get_nc(*_args, **_kwargs):
    if "nc" not in _NC_CACHE:
        _NC_CACHE["nc"] = build_nc()
    return _NC_CACHE["nc"]


def pack_inputs(input, weight, bias):
    """Host-side prepack: returns per-core in_maps."""
    x = np.ascontiguousarray(input, dtype=np.int32)
    # [core, img, ch, g, b, par, w] -> [core, g, par, ch, b, img, w]
    xr = (x.astype(np.int16) - IN_ZP).astype(BF16)
    xr = xr.reshape(N_CORES, N_IMG, C, N_GROUPS, 4, 2, W)
    xr = np.ascontiguousarray(xr.transpose(0, 3, 5, 2, 4, 1, 6))
    xp = xr.reshape(N_CORES, N_GROUPS, 128, GCOL)

    wf = weight.astype(np.float32) - W_ZP            # [O, I, kh, kw]
    # wp[p, (kh*3+kw)*64 + o] = wf[o, p%64, kh, kw], both partition halves
    wpk = np.zeros((128, 9 * 64 + 6 * 128), np.float32)
    for kh in range(3):
        for kw in range(3):
            t = kh * 3 + kw
            blk = wf[:, :, kh, kw].T                 # [ci, o]
            wpk[0:64, t * 64:(t + 1) * 64] = blk
            wpk[64:128, t * 64:(t + 1) * 64] = blk
    # v2-style full-array tiles for the tail pair:
    # lhs[t][par*64+c][r*64+o] = wf[o, c, 2*j2+par-r, kw]
    lhs = np.zeros((6, 128, 128), np.float32)
    for j2 in range(2):
        for kw in range(3):
            t = j2 * 3 + kw
            for par in range(2):
                for r in range(2):
                    kh = 2 * j2 + par - r
                    if 0 <= kh <= 2:
                        lhs[t, par * 64:par * 64 + 64,
                            r * 64:r * 64 + 64] = wf[:, :, kh, kw].T
    wpk[:, 576:] = lhs.transpose(1, 0, 2).reshape(128, 768)
    wpk = wpk.astype(BF16)

    bpk = np.concatenate([bias, bias]).astype(np.float32).reshape(128, 1)

    return [{"xp": np.ascontiguousarray(xp[i]), "wp": wpk, "bp": bpk}
            for i in range(N_CORES)]


def unpack_output(yp):
    """[N_UNITS, 128, 2*BLK] int16 -> [N_IMG, CO, OH, OW] int32.

    yp[u, cg*64+co, bank*BLK + img*W + w] = out row 4u + 2*cg + bank.
    """
    a = yp.reshape(N_UNITS, 2, CO, 2, N_IMG, W)      # [u, cg, co, bank, img, w]
    a = a.transpose(4, 2, 0, 1, 3, 5)                # [img, co, u, cg, bank, w]
    a = a.reshape(N_IMG, CO, N_UNITS * 4, W)
    return a[:, :, :OH, :OW].astype(np.int32)


def run_sharded(nc, input, weight, bias, n_img=N_IMG, **kwargs):
    in_maps = pack_inputs(input, weight, bias)
    res = run_bass_kernel_spmd(nc, in_maps, list(range(N_CORES)), **kwargs)
    out = np.concatenate([unpack_output(r["yp"]) for r in res.results],
                         axis=0)
    return out, res


def kernel(input, weight, bias):
    nc = get_nc()
    out, _ = run_sharded(nc, input, weight, bias)
    return out
